# revision 13
# baseline (speedup 1.0000x reference)
"""Distributed Trainium2 kernel for AssociativeSparseDistributedMemory.get_cliques.

Reference (B=128, INPUT=1024, VCAP=32768, K=32, ACAP=4096, K2=32):
  scores  = keys @ value_proj.T;  idx1 = top_k(scores, 32)
  p       = clique_encoder[idx1].sum(1)   (scale+normalize skipped: a positive
                                           per-row scale never changes a top-k set)
  scores2 = p @ assoc_proj.T;     idx2 = top_k(scores2, 32)
  out     = assoc_mem_value[idx2].sum(1)

Distribution over 8 cores (core m):
  B': value_proj rows [4096m, 4096(m+1)) -> score chunk [128, 4096], computed
      512 columns at a time; as each 512-chunk lands, DVE finds its top-32
      values (max8/match_replace8) AND their positions (find_index8), overlap
      with the next chunk's matmul.
  C': core-level merge of 8x32 candidates -> core top-32 (values + global
      indices via a masked-index top-k: sel = top32(mask ? index : -1)).
  D': AllGather [128, 64] (vals|idx) -> global merge -> the exact global
      top-32 indices [128, 32] on every core, plus threshold t32.
  E': indices -> int16 in the DGE wrapped layout (via a DRAM bounce with an
      einops-rearranged access pattern), dma_gather pulls the 4096 selected
      rows of the column-sharded clique_encoder (E[:, 512m:512(m+1)], 2KB
      rows); sum over the 32 slots -> p chunk [128, 512]; AllGather p.
  K : scores2 chunk = p @ assoc_proj[512m:512(m+1)].T (fp32, PE-transposed p).
  L': local top-32 values, AllGather, merge -> t32_2; mask2 = s2 >= t32_2;
      AllGather mask2 -> full selection w2 [128, 4096].
  Q : out chunk = w2 @ M[:, 4096m:4096(m+1)) in BF16 (selection already done;
      0/1 weights exact in bf16, table quantization well under tolerance).
      The dense read equals gather traffic here (4096 rows, 4096 draws).
"""

import numpy as np

B = 128
INPUT = 1024
VCAP = 32768
ACAP = 4096
K = 32
NCORES = 8
VSH = VCAP // NCORES      # 4096 value rows per core
ASH = ACAP // NCORES      # 512 assoc rows per core

_CACHE = {}

NEG = -1e30


def _build():
    import concourse.bass as bass
    import concourse.mybir as mybir
    import concourse.tile as tile
    from concourse import bacc
    from concourse.masks import make_identity

    f32 = mybir.dt.float32
    bf16 = mybir.dt.bfloat16
    i16 = mybir.dt.int16
    u16 = mybir.dt.uint16
    u8 = mybir.dt.uint8
    Alu = mybir.AluOpType

    nc = bacc.Bacc("TRN2", target_bir_lowering=False, debug=False,
                   num_devices=NCORES)

    # ---- kernel I/O ----
    keysT_d = nc.dram_tensor("keysT", [INPUT, B], f32, kind="ExternalInput")
    vpTt_d = nc.dram_tensor("vpTt", [8, 8, 128, 512], f32, kind="ExternalInput")
    Ecol_d = nc.dram_tensor("Ecol", [VCAP, ASH], f32, kind="ExternalInput")
    apT_d = nc.dram_tensor("apT", [ACAP, ASH], f32, kind="ExternalInput")
    Mb_d = nc.dram_tensor("Mb", [ACAP, VSH], bf16, kind="ExternalInput")
    rbase_d = nc.dram_tensor("rbase", [B, 1], f32, kind="ExternalInput")
    out_d = nc.dram_tensor("out", [B, VSH], f32, kind="ExternalOutput")

    # ---- internal DRAM ----
    cand1_in = nc.dram_tensor("cand1_in", [B, K], f32)
    cand1_out = nc.dram_tensor("cand1_out", [B * NCORES, K], f32,
                               addr_space="Shared")
    idxag_in = nc.dram_tensor("idxag_in", [B, K], f32)
    idxag_out = nc.dram_tensor("idxag_out", [B * NCORES, K], f32,
                               addr_space="Shared")
    # [c, p, k, s0]: gidx_tmp[c, p, k, s0] = global_idx[16*s0 + p, k]; the DGE
    # wrapped index layout reads it as "(c p) (k s0)".
    gidx_tmp = nc.dram_tensor("gidx_tmp", [NCORES, 16, K, 8], i16)
    pag_in = nc.dram_tensor("pag_in", [B, ASH], f32)
    pag_out = nc.dram_tensor("pag_out", [B * NCORES, ASH], f32,
                             addr_space="Shared")
    cand2_in = nc.dram_tensor("cand2_in", [B, K], f32)
    cand2_out = nc.dram_tensor("cand2_out", [B * NCORES, K], f32,
                               addr_space="Shared")
    m2_in = nc.dram_tensor("m2_in", [B, ASH], f32)
    m2_out = nc.dram_tensor("m2_out", [B * NCORES, ASH], f32,
                            addr_space="Shared")

    RG = [list(range(NCORES))]

    with tile.TileContext(nc) as tc:
        with (
            tc.tile_pool(name="const", bufs=1) as constp,
            tc.tile_pool(name="big", bufs=1) as bigp,
            tc.tile_pool(name="small", bufs=1) as smallp,
            tc.tile_pool(name="chk", bufs=3) as chkp,
            tc.tile_pool(name="rhsB", bufs=6) as rhsBp,
            tc.tile_pool(name="rhsK", bufs=3) as rhsKp,
            tc.tile_pool(name="rhsQ", bufs=4) as rhsQp,
        ):
            psA_cm = tc.tile_pool(name="psA", bufs=2, space="PSUM")
            psA = psA_cm.__enter__()
            ident = constp.tile([128, 128], f32)
            make_identity(nc, ident[:, :])
            keysT_sb = constp.tile([128, 8, 128], f32)
            for k in range(8):
                nc.sync.dma_start(out=keysT_sb[:, k, :],
                                  in_=keysT_d[k * 128:(k + 1) * 128, :])
            rbase = constp.tile([B, 1], f32)
            nc.sync.dma_start(out=rbase[:, :], in_=rbase_d[:, :])

            # ---- stage B': scores chunks + pipelined per-chunk top-32 ----
            vals256 = smallp.tile([B, 8, K], f32)    # per-chunk top-32 values
            cidx256 = smallp.tile([B, 8, K], f32)    # their global indices
            for n in range(8):
                ps = psA.tile([128, 512], f32, tag="ps", name=f"psB{n}")
                for k in range(8):
                    rhs = rhsBp.tile([128, 512], f32, tag="rhs", name=f"rB{n}_{k}")
                    nc.sync.dma_start(out=rhs[:, :], in_=vpTt_d[n, k])
                    nc.tensor.matmul(ps[:, :], keysT_sb[:, k, :], rhs[:, :],
                                     start=(k == 0), stop=(k == 7))
                schunk = chkp.tile([B, 512], f32, tag="schunk", name=f"sch{n}")
                nc.scalar.copy(schunk[:, :], ps[:, :])
                scr = chkp.tile([B, 512], f32, tag="scr", name=f"scr{n}")
                idxn = chkp.tile([B, K], u16, tag="idxn", name=f"idxn{n}")
                for r in range(4):
                    s = schunk if r == 0 else scr
                    nc.vector.max(out=vals256[:, n, r * 8:(r + 1) * 8], in_=s[:, :])
                    nc.vector.max_index(out=idxn[:, r * 8:(r + 1) * 8],
                                        in_max=vals256[:, n, r * 8:(r + 1) * 8],
                                        in_values=schunk[:, :])
                    nc.vector.match_replace(
                        out=scr[:, :],
                        in_to_replace=vals256[:, n, r * 8:(r + 1) * 8],
                        in_values=s[:, :], imm_value=NEG)
                # global index = pos + rank_base + n*512
                nc.vector.tensor_scalar(
                    out=cidx256[:, n, :], in0=idxn[:, :], scalar1=rbase[:, :],
                    scalar2=float(n * 512), op0=Alu.add, op1=Alu.add)

            # ---- stage C': core-level value merge ----
            def topk32(vals, width, pool, pref):
                """mv [B, 32] = top-32 values of vals [B, width] (descending)."""
                mv = pool.tile([B, K], f32, name=f"{pref}_mv", tag=f"{pref}_mv")
                ms = pool.tile([B, width], f32, name=f"{pref}_ms", tag=f"{pref}_ms")
                for r in range(4):
                    s = vals if r == 0 else ms[:, :]
                    nc.vector.max(out=mv[:, r * 8:(r + 1) * 8], in_=s)
                    nc.vector.match_replace(
                        out=ms[:, :], in_to_replace=mv[:, r * 8:(r + 1) * 8],
                        in_values=s, imm_value=NEG)
                return mv

            vals256f = vals256[:, :, :].rearrange("b e k -> b (e k)")
            cmv = topk32(vals256f, 8 * K, smallp, "cm")

            # ---- stage D1: AllGather core top-32 values, derive global t32 ----
            nc.sync.dma_start(out=cand1_in[:, :], in_=cmv[:, :])
            nc.gpsimd.collective_compute(
                "AllGather", Alu.bypass, replica_groups=RG,
                ins=[cand1_in.ap().opt()], outs=[cand1_out.ap().opt()])
            gvals = smallp.tile([B, NCORES, K], f32)
            for r in range(NCORES):
                nc.sync.dma_start(out=gvals[:, r, :],
                                  in_=cand1_out[r * B:(r + 1) * B, :])
            gmv = topk32(gvals[:, :, :].rearrange("b e k -> b (e k)"),
                         NCORES * K, smallp, "gm")

            # ---- stage E1: local index extraction (pairs aligned per chunk) ----
            msk = smallp.tile([B, 8 * K], u8)
            nc.vector.tensor_scalar(out=msk[:, :], in0=vals256f,
                                    scalar1=gmv[:, K - 1:K], scalar2=None,
                                    op0=Alu.is_ge)
            mi = smallp.tile([B, 8 * K], f32)
            nc.vector.memset(mi[:, :], -1.0)
            nc.vector.copy_predicated(
                out=mi[:, :], mask=msk[:, :],
                data=cidx256[:, :, :].rearrange("b e k -> b (e k)"))
            lidx = topk32(mi[:, :], 8 * K, smallp, "li")

            # ---- stage D2: AllGather index lists, final top-32 -> giv ----
            nc.sync.dma_start(out=idxag_in[:, :], in_=lidx[:, :])
            nc.gpsimd.collective_compute(
                "AllGather", Alu.bypass, replica_groups=RG,
                ins=[idxag_in.ap().opt()], outs=[idxag_out.ap().opt()])
            gidxall = smallp.tile([B, NCORES, K], f32)
            for r in range(NCORES):
                nc.sync.dma_start(out=gidxall[:, r, :],
                                  in_=idxag_out[r * B:(r + 1) * B, :])
            giv = topk32(gidxall[:, :, :].rearrange("b e k -> b (e k)"),
                         NCORES * K, smallp, "gi")

            # ---- stage E': global idx -> int16 wrapped layout -> dma_gather ----
            gidx16 = smallp.tile([B, K], i16)
            nc.vector.tensor_copy(gidx16[:, :], giv[:, :])
            for c in range(NCORES):
                nc.sync.dma_start(out=gidx_tmp[c].rearrange("p k s0 -> s0 p k"),
                                  in_=gidx16[:, :])
            idxs16 = smallp.tile([128, 256], i16)   # 4096 idxs / 16 lanes
            nc.sync.dma_start(
                out=idxs16[:, :],
                in_=gidx_tmp.ap().rearrange("c p k s0 -> (c p) (k s0)"))
            gath = bigp.tile([128, K, ASH], f32, tag="gath")   # 8 MB
            # SWDGE ring holds 128 descriptors/engine; one gather emits
            # num_idxs/16+1, so split 4096 indices into 4 calls of 1024.
            for j in range(4):
                nc.gpsimd.dma_gather(
                    out_ap=gath[:, j * 8:(j + 1) * 8, :], in_ap=Ecol_d.ap(),
                    idxs_ap=idxs16[:, j * 64:(j + 1) * 64],
                    num_idxs=1024, num_idxs_reg=1024, elem_size=ASH)
            p_chunk = smallp.tile([B, ASH], f32)
            nc.vector.tensor_reduce(
                out=p_chunk[:, :], in_=gath[:, :, :].rearrange("b k c -> b c k"),
                axis=mybir.AxisListType.X, op=Alu.add)

            # ---- AllGather p ----
            nc.sync.dma_start(out=pag_in[:, :], in_=p_chunk[:, :])
            nc.gpsimd.collective_compute(
                "AllGather", Alu.bypass, replica_groups=RG,
                ins=[pag_in.ap().opt()], outs=[pag_out.ap().opt()])
            p_full = bigp.tile([B, ACAP], f32, tag="A")
            for r in range(NCORES):
                nc.sync.dma_start(out=p_full[:, r * ASH:(r + 1) * ASH],
                                  in_=pag_out[r * B:(r + 1) * B, :])

            # ---- stage J: pT tiles ----
            pT = bigp.tile([128, 32, 128], f32, tag="B")
            for t in range(32):
                pt = psA.tile([128, 128], f32, tag="ps", name=f"ptJ{t}")
                nc.tensor.transpose(pt[:, :], p_full[:, t * 128:(t + 1) * 128],
                                    ident[:, :])
                nc.scalar.copy(pT[:, t, :], pt[:, :])

            # ---- stage K: scores2 chunk (fp32) ----
            s2 = smallp.tile([B, ASH], f32, tag="s2")
            psK = psA.tile([128, 512], f32, tag="ps", name="psK")
            for k in range(32):
                rhs = rhsKp.tile([128, ASH], f32, tag="rhs", name=f"rK{k}")
                nc.sync.dma_start(out=rhs[:, :],
                                  in_=apT_d[k * 128:(k + 1) * 128, :])
                nc.tensor.matmul(psK[:, :], pT[:, k, :], rhs[:, :],
                                 start=(k == 0), stop=(k == 31))
            nc.scalar.copy(s2[:, :], psK[:, :])

            # ---- stage L/M: local top-32 values, AG, merge -> t32_2 ----
            scr2 = smallp.tile([B, ASH], f32, tag="scr2")
            cand2 = smallp.tile([B, K], f32, tag="c2")
            for r in range(4):
                s = s2 if r == 0 else scr2
                nc.vector.max(out=cand2[:, r * 8:(r + 1) * 8], in_=s[:, :])
                nc.vector.match_replace(
                    out=scr2[:, :], in_to_replace=cand2[:, r * 8:(r + 1) * 8],
                    in_values=s[:, :], imm_value=NEG)
            nc.sync.dma_start(out=cand2_in[:, :], in_=cand2[:, :])
            nc.gpsimd.collective_compute(
                "AllGather", Alu.bypass, replica_groups=RG,
                ins=[cand2_in.ap().opt()], outs=[cand2_out.ap().opt()])
            cands2 = smallp.tile([B, NCORES * K], f32, tag="cs2")
            for r in range(NCORES):
                nc.sync.dma_start(out=cands2[:, r * K:(r + 1) * K],
                                  in_=cand2_out[r * B:(r + 1) * B, :])
            mcand2 = smallp.tile([B, K], f32, tag="mc2")
            mscr2 = smallp.tile([B, NCORES * K], f32, tag="ms2")
            for r in range(4):
                s = cands2 if r == 0 else mscr2
                nc.vector.max(out=mcand2[:, r * 8:(r + 1) * 8], in_=s[:, :])
                nc.vector.match_replace(
                    out=mscr2[:, :], in_to_replace=mcand2[:, r * 8:(r + 1) * 8],
                    in_values=s[:, :], imm_value=NEG)

            # ---- stage N/O: mask2, AllGather -> w2 ----
            mask2 = smallp.tile([B, ASH], f32, tag="m2")
            nc.vector.tensor_scalar(
                out=mask2[:, :], in0=s2[:, :], scalar1=mcand2[:, K - 1:K],
                scalar2=None, op0=Alu.is_ge)
            nc.sync.dma_start(out=m2_in[:, :], in_=mask2[:, :])
            nc.gpsimd.collective_compute(
                "AllGather", Alu.bypass, replica_groups=RG,
                ins=[m2_in.ap().opt()], outs=[m2_out.ap().opt()])
            w2 = bigp.tile([B, ACAP], f32, tag="A")
            for r in range(NCORES):
                nc.sync.dma_start(out=w2[:, r * ASH:(r + 1) * ASH],
                                  in_=m2_out[r * B:(r + 1) * B, :])

            # ---- stage P: w2T tiles (bf16) ----
            w2T = bigp.tile([128, 32, 128], bf16, tag="w2T")
            for t in range(32):
                pt = psA.tile([128, 128], f32, tag="ps", name=f"ptP{t}")
                nc.tensor.transpose(pt[:, :], w2[:, t * 128:(t + 1) * 128],
                                    ident[:, :])
                nc.scalar.copy(w2T[:, t, :], pt[:, :])

            # ---- stage Q: out chunk = w2 @ M_shard (bf16) ----
            psA_cm.__exit__(None, None, None)
            psQp_cm = tc.tile_pool(name="psQ", bufs=8, space="PSUM")
            psQp = psQp_cm.__enter__()
            out_sb = bigp.tile([B, VSH], f32, tag="B")
            psQ = [psQp.tile([128, 512], f32, tag="pq", name=f"psQ{n}")
                   for n in range(8)]
            for k in range(32):
                rhs = rhsQp.tile([128, VSH], bf16, tag="rhs", name=f"rQ{k}")
                nc.sync.dma_start(out=rhs[:, :],
                                  in_=Mb_d[k * 128:(k + 1) * 128, :])
                for n in range(8):
                    nc.tensor.matmul(psQ[n][:, :], w2T[:, k, :],
                                     rhs[:, n * 512:(n + 1) * 512],
                                     start=(k == 0), stop=(k == 31))
            for n in range(8):
                nc.scalar.copy(out_sb[:, n * 512:(n + 1) * 512], psQ[n][:, :])
            nc.sync.dma_start(out=out_d[:, :], in_=out_sb[:, :])
            psQp_cm.__exit__(None, None, None)

    nc.compile()
    return nc


def get_nc():
    if "nc" not in _CACHE:
        _CACHE["nc"] = _build()
    return _CACHE["nc"]


def make_in_maps(keys, value_proj, clique_encoder, assoc_proj, assoc_mem_value):
    import ml_dtypes
    keysT = np.ascontiguousarray(np.asarray(keys).T.astype(np.float32))
    value_proj = np.asarray(value_proj).astype(np.float32)
    clique_encoder = np.asarray(clique_encoder).astype(np.float32)
    assoc_proj = np.asarray(assoc_proj).astype(np.float32)
    Mb_full = np.asarray(assoc_mem_value).astype(ml_dtypes.bfloat16)
    in_maps = []
    for m in range(NCORES):
        vpT = value_proj[m * VSH:(m + 1) * VSH, :].T       # [1024, 4096]
        vpTt = np.empty((8, 8, 128, 512), np.float32)
        for n in range(8):
            for k in range(8):
                vpTt[n, k] = vpT[k * 128:(k + 1) * 128, n * 512:(n + 1) * 512]
        in_maps.append({
            "keysT": keysT,
            "vpTt": vpTt,
            "Ecol": np.ascontiguousarray(
                clique_encoder[:, m * ASH:(m + 1) * ASH]),
            "apT": np.ascontiguousarray(
                assoc_proj[m * ASH:(m + 1) * ASH, :].T),
            "Mb": np.ascontiguousarray(Mb_full[:, m * VSH:(m + 1) * VSH]),
            "rbase": np.full((B, 1), m * VSH, np.float32),
        })
    return in_maps


def kernel(keys, value_proj, clique_encoder, assoc_proj, assoc_mem_value,
           **run_kwargs):
    from concourse.bass_utils import run_bass_kernel_spmd

    nc = get_nc()
    in_maps = make_in_maps(keys, value_proj, clique_encoder, assoc_proj,
                           assoc_mem_value)
    res = run_bass_kernel_spmd(nc, in_maps, core_ids=list(range(NCORES)),
                               **run_kwargs)
    out = np.concatenate([np.asarray(res.results[m]["out"])
                          for m in range(NCORES)], axis=1)
    _CACHE["last_result"] = res
    return out
